# revision 22
# baseline (speedup 1.0000x reference)
"""Trainium2 Bass kernel for AuxiliaryMultiHeadedAttention.

Reference computation (B=4, L=2048, H=256, NH=8, DH=32):
    kb   = split_heads(k_b @ Wb.T + bb)
    corr = (qh @ kh^T + qh @ kb^T) / sqrt(DH) * scale_w[h, q]
    corr = where(mask==0, -1e9, corr);  prob = softmax(corr)
    out  = merge_heads(prob @ vh) @ Ww.T + bw

Kernel strategy (8 NeuronCores):
    Shard (batch, query-half): core c -> batch c//2, queries (c%2)*1024..+1024.
    Host marshals layouts (pre-transposed bf16 k^T / k_b^T / W^T, the
    mask-interleaved V operand, scale_w chunks) so the device does zero
    layout shuffling; device computes both GEMMs, QK^T, softmax, PV and
    the output projection.
    Each core:
      keffT = (k + k_b @ Wb.T + bb)^T  [dims, keys]  bf16 (dual QK^T folded)
      qsT   = (q * scale_w/sqrt(DH))^T via DMA-xbar  [dims, queries] bf16
      S^T   = keffT_h^T @ qsT_h  (bf16 MMs, 2 heads row-tiled, fp32 psum;
              kc pairs alternate PE row groups via 64-row-shifted copies)
      P^T   = exp(S^T): split between ACT (exact exp, bf16 out) and DVE
              (Schraudolph: int16(A*x+B) bitcast to bf16, one tensor_scalar)
      PV with lhsT [m|v_h] / [v_h|m] (m = mask: masks numerator and
              denominator) -> psum rows [den0|O0|O1|den1]
      hidT  = O * recip(den)  (full-partition recip + mul, DMA realign)
      out   = hidT^T @ WwT + bw (bias via rank-1 matmul), query-half-outer
              loop lets the first half's projection overlap the second half.
    Host concatenates the 8 [1024, 256] slices.
"""

import sys

if "/opt/trn_rl_repo" not in sys.path:
    sys.path.insert(0, "/opt/trn_rl_repo")

import math

import numpy as np

B, L, H, NH, DH = 4, 2048, 256, 8, 32
LQ = 1024  # queries per core
NCORES = 8
ISQ = 1.0 / math.sqrt(DH)

# Schraudolph exp for bf16 target: bf16bits(exp(x)) ~ int16(A16*x + B16)
A16 = 128.0 / math.log(2.0)
C_OFF = 5.5
B16 = 127.0 * 128.0 - C_OFF


def _build():
    import concourse.bass as bass  # noqa: F401
    import concourse.mybir as mybir
    import concourse.tile as tile
    from concourse import bacc

    f32 = mybir.dt.float32
    i16 = mybir.dt.int16
    bf16 = mybir.dt.bfloat16
    Exp = mybir.ActivationFunctionType.Exp

    nc = bacc.Bacc("TRN2", target_bir_lowering=False, debug=False, num_devices=NCORES)

    qsT_d = nc.dram_tensor("qsT", [H, LQ], bf16, kind="ExternalInput")
    qsT2_d = nc.dram_tensor("qsT2", [H, LQ], bf16, kind="ExternalInput")
    kT_d = nc.dram_tensor("kT", [H, L], bf16, kind="ExternalInput")
    kbT_d = nc.dram_tensor("kbT", [H, L], bf16, kind="ExternalInput")
    vmm_d = nc.dram_tensor("vmm", [128, 8192], bf16, kind="ExternalInput")
    WbT_d = nc.dram_tensor("WbT", [H, H], bf16, kind="ExternalInput")
    WwT_d = nc.dram_tensor("WwT", [H, H], bf16, kind="ExternalInput")
    bwb_d = nc.dram_tensor("bwb", [1, H], bf16, kind="ExternalInput")
    ones_d = nc.dram_tensor("ones", [1, L], bf16, kind="ExternalInput")
    out_d = nc.dram_tensor("out", [LQ, H], f32, kind="ExternalOutput")

    copy_flip = [0]

    with tile.TileContext(nc) as tc:
        with (
            tc.tile_pool(name="persist", bufs=1) as pp,
            tc.tile_pool(name="pt", bufs=4) as ptp,
            tc.tile_pool(name="small", bufs=2) as smp,
        ):
            # ---------------- persistent SBUF tensors ----------------
            keffT = [pp.tile([128, L], bf16, tag=f"keffT{g}", name=f"keffT{g}")
                     for g in range(2)]
            keffT2 = [pp.tile([128, L], bf16, tag=f"keffT2_{g}",
                              name=f"keffT2_{g}") for g in range(2)]
            qsT = [pp.tile([128, LQ], bf16, tag=f"qsT{g}", name=f"qsT{g}")
                   for g in range(2)]
            qsT2 = [pp.tile([128, LQ], bf16, tag=f"qsT2_{g}", name=f"qsT2_{g}")
                    for g in range(2)]
            # per (key-chunk, head): [m|v_h] (h even) / [v_h|m] (h odd);
            # m = mask column (masks numerator and denominator)
            vmm = pp.tile([128, 16 * NH * 64], bf16, tag="vmm")
            hidT = [pp.tile([128, LQ], bf16, tag=f"hidT{g}", name=f"hidT{g}")
                    for g in range(2)]
            outsb = pp.tile([128, 8 * H], f32, tag="outsb")

            kbTt = [pp.tile([128, L], bf16, tag=f"kbTt{e}", name=f"kbTt{e}")
                    for e in range(2)]
            kTt = [pp.tile([128, L], bf16, tag=f"kTt{e}", name=f"kTt{e}")
                   for e in range(2)]
            WbTt = [pp.tile([128, H], bf16, tag=f"WbTt{e}", name=f"WbTt{e}")
                    for e in range(2)]
            WwTt = [pp.tile([128, H], bf16, tag=f"WwTt{g}", name=f"WwTt{g}")
                    for g in range(2)]
            bwb = pp.tile([1, H], bf16, tag="bwb")
            ones = pp.tile([1, L], bf16, tag="ones")
            wsc = pp.tile([128, 256], bf16, tag="wsc")
            zf = pp.tile([128, 64], bf16, tag="zf")

            def pcopy(dst, src):
                # alternate psum->sbuf evacuation between DVE and ACT
                if copy_flip[0] % 2 == 0:
                    nc.vector.tensor_copy(dst, src)
                else:
                    nc.scalar.copy(dst, src)
                copy_flip[0] += 1

            # ---------------- staging loads ----------------
            # 3 DMA queues (sync/scalar HWDGE + gpsimd SWDGE), ordered by
            # criticality; vmm (largest) is chunked so it trails into the
            # main loop (PV consumes it in kc order).
            vmv = vmm.rearrange("p (c f) -> p c f", c=4)
            # warm-up operand needs no load
            nc.vector.memset(wsc, 0.5)
            nc.vector.memset(zf, 0.0)
            # gpsimd: kbT ec0 (keff critical path); keffT2 shifts + vmm
            # kc 8-15 queued after the keff GEMM below
            nc.gpsimd.dma_start(out=kbTt[0], in_=kbT_d[0:128, :])
            # scalar: kbT ec1, q tiles, vmm kc 0-7 trailing
            nc.scalar.dma_start(out=kbTt[1], in_=kbT_d[128:256, :])
            for dc in range(2):
                nc.scalar.dma_start(out=qsT[dc],
                                    in_=qsT_d[dc * 128:(dc + 1) * 128, :])
                nc.scalar.dma_start(out=qsT2[dc],
                                    in_=qsT2_d[dc * 128:(dc + 1) * 128, :])
            nc.scalar.dma_start(out=vmv[:, 0, :], in_=vmm_d[:, 0:2048])
            nc.scalar.dma_start(out=vmv[:, 1, :], in_=vmm_d[:, 2048:4096])
            # sync: keff deps first, tail-only tensors last
            nc.sync.dma_start(out=WbTt[0], in_=WbT_d[0:128, :])
            nc.sync.dma_start(out=WbTt[1], in_=WbT_d[128:256, :])
            for ec in range(2):
                nc.sync.dma_start(out=kTt[ec],
                                  in_=kT_d[ec * 128:(ec + 1) * 128, :])
            nc.sync.dma_start(out=WwTt[0], in_=WwT_d[0:128, :])
            nc.sync.dma_start(out=WwTt[1], in_=WwT_d[128:256, :])
            nc.sync.dma_start(out=bwb, in_=bwb_d[:, :])
            nc.sync.dma_start(out=ones, in_=ones_d[:, :])

            # ---------------- keff GEMM ----------------
            # bb is pre-added into kT on the host, so no bias matmuls:
            # keffT = WbT.T @ kbT (psum) + (k.T + bb) via fused evac add.
            with tc.tile_pool(name="pkeff", bufs=4, space="PSUM") as pkf:
                # PE warm-up burst: dense dummy matmuls on a memset scratch
                # release the HAM clock gate (~3.4us of sustained PE
                # activity) before keff/QK need full rate.
                for w in range(32):
                    pwt = (w % 8) // 2
                    pw = pkf.tile([128, 512], f32, tag="pk", name="pkw")
                    nc.tensor.matmul(pw[:, 0:256],
                                     lhsT=wsc[:, 0:128],
                                     rhs=wsc[:, 0:256],
                                     start=True, stop=True,
                                     skip_group_check=True)
                # keff GEMM, one psum tile per 512-key chunk so the DVE
                # evacuation never serializes the PE stream (tile-granular
                # dependency tracking)
                for dc in range(2):
                    for ns in range(4):
                        co = slice(ns * 512, (ns + 1) * 512)
                        pk = pkf.tile([128, 512], f32, tag="pk",
                                      name=f"pk{dc}_{ns}")
                        for ec in range(2):
                            nc.tensor.matmul(
                                pk,
                                lhsT=WbTt[ec][:, dc * 128:(dc + 1) * 128],
                                rhs=kbTt[ec][:, co],
                                start=(ec == 0), stop=(ec == 1))
                        # evac with fused +(k+bb) add; 64-row shift per ns
                        # so QK for kc 4ns.. can start immediately
                        nc.vector.tensor_add(keffT[dc][:, co], pk,
                                             kTt[dc][:, co])
                        nc.gpsimd.dma_start(out=keffT2[dc][0:64, co],
                                            in_=keffT[dc][64:128, co])
                        nc.gpsimd.dma_start(out=keffT2[dc][64:128, co],
                                            in_=keffT[dc][0:64, co])
                # vmm kc 8-15 trail on the gpsimd queue
                nc.gpsimd.dma_start(out=vmv[:, 2, :], in_=vmm_d[:, 4096:6144])
                nc.gpsimd.dma_start(out=vmv[:, 3, :], in_=vmm_d[:, 6144:8192])

            # ---------------- main attention loop ----------------
            # query-half outer so the first half's output projection can
            # overlap the second half's attention.
            # group g: heads (2g, 2g+1); chunk ch = g//2.
            # kc processed in pairs with alternating PE row groups (via the
            # 64-row-shifted tile copies): the pair's 4 QK matmuls occupy 4
            # distinct 32-row groups and stream concurrently.
            with (
                tc.tile_pool(name="pst", bufs=3, space="PSUM") as pst,
                tc.tile_pool(name="ppv", bufs=2, space="PSUM") as ppv,
            ):
                def out_proj(mq):
                    # out[mq] = hidT^T @ WwT + bw (bias via rank-1 matmul)
                    po = pst.tile([128, 1024], f32, tag="st", name="po")
                    nc.tensor.matmul(po[:, 0:256],
                                     lhsT=ones[0:1, mq * 128:(mq + 1) * 128],
                                     rhs=bwb[0:1, :],
                                     start=True, stop=False)
                    for gg in range(2):
                        nc.tensor.matmul(
                            po[:, 0:256],
                            lhsT=hidT[gg][:, mq * 128:(mq + 1) * 128],
                            rhs=WwTt[gg],
                            start=False, stop=(gg == 1))
                    pcopy(outsb[:, mq * H:(mq + 1) * H], po[:, 0:256])
                    if mq % 2 == 1:
                        cs2 = slice(mq - 1, mq + 1)
                        nc.sync.dma_start(
                            out=out_d.rearrange("(c p) e -> p c e",
                                                p=128)[:, cs2, :],
                            in_=outsb.rearrange("p (c e) -> p c e",
                                                c=8)[:, cs2, :])

                def pv_mms(pv, g, kc2, pt, last):
                    # PV: h even lhsT=[m|v] -> rows [den|O];
                    #     h odd  lhsT=[v|m] -> rows [O|den]
                    for t in range(2):
                        h = 2 * g + t
                        nc.tensor.matmul(
                            pv[64 * t:64 * t + 64, :],
                            lhsT=vmm[:, (kc2 * NH + h) * 64:
                                     (kc2 * NH + h) * 64 + 64],
                            rhs=pt[:, t * 512:(t + 1) * 512],
                            tile_position=(0, 64 * t),
                            start=(kc2 == 0), stop=last,
                            skip_group_check=True)

                for qh in range(2):
                    for g in range(4):
                        ch = g // 2
                        pv = ppv.tile([128, 512], f32, tag="pv",
                                      name=f"pv{qh}_{g}")
                        # software-pipelined by two kc: PV(k) is issued
                        # after QK(k+2)+exp(k+2) so the (dependency-stalled)
                        # PV matmul never head-of-line-blocks later QKs
                        # in the PE's in-order queue.
                        pend = []
                        for kc2 in range(16):
                            idx = ((qh * 4 + g) * 16 + kc2) // 2
                            par = kc2 % 2
                            kket = keffT[ch] if par == 0 else keffT2[ch]
                            qqt = qsT[ch] if par == 0 else qsT2[ch]
                            rbase = ((g % 2) * 64 if par == 0
                                     else (1 - g % 2) * 64)
                            st = pst.tile([128, 1024], f32, tag="st",
                                          name=f"st{par}")
                            for t in range(2):
                                ro = rbase + t * 32
                                nc.tensor.matmul(
                                    st[:, t * 512:(t + 1) * 512],
                                    lhsT=kket[ro:ro + 32,
                                              kc2 * 128:(kc2 + 1) * 128],
                                    rhs=qqt[ro:ro + 32,
                                            qh * 512:(qh + 1) * 512],
                                    tile_position=(ro, 0),
                                    start=True, stop=True)
                            # exp: odd kc -> ACT; even kc -> DVE
                            # (Schraudolph), except every 16th even also
                            # ACT (balance: 72 ACT / 56 DVE)
                            if par == 1 or idx % 16 == 5:
                                pt = ptp.tile([128, 1024], bf16,
                                              tag="ptA", name="ptA")
                                nc.scalar.activation(pt, st, Exp)
                            else:
                                pti = ptp.tile([128, 1024], i16,
                                               tag="ptD", name="ptD")
                                nc.vector.tensor_scalar(
                                    out=pti, in0=st, scalar1=A16,
                                    scalar2=B16,
                                    op0=mybir.AluOpType.mult,
                                    op1=mybir.AluOpType.add)
                                pt = pti.bitcast(bf16)
                            pend.append((kc2, pt))
                            if len(pend) > 2:
                                k0, p0 = pend.pop(0)
                                pv_mms(pv, g, k0, p0, False)
                                # zero-accumulating filler matmul: keeps
                                # the PE fed when the HAM clock gate is
                                # cold so it re-arms to 2.4 GHz (adds 0)
                                nc.tensor.matmul(
                                    pv[0:64, 0:256], lhsT=zf,
                                    rhs=keffT[0][:, 0:256],
                                    tile_position=(0, 0),
                                    start=False, stop=False,
                                    skip_group_check=True)
                            # interleave qh=0's output projection into
                            # qh=1/g=0 so only mq 4-7 remain in the tail
                            if qh == 1 and g == 0 and kc2 in (4, 6, 8, 10):
                                out_proj((kc2 - 4) // 2)
                        for k0, p0 in pend:
                            pv_mms(pv, g, k0, p0, k0 == 15)
                        # normalize: pv rows = [den0 | O0 | O1 | den1].
                        # Full 128-partition ops; unused lanes compute
                        # garbage, unread.
                        ntmp = smp.tile([128, 512], f32, tag="ntmp",
                                        name="ntmp")
                        nc.vector.reciprocal_approx_fast(ntmp, pv)
                        rtl = smp.tile([128, 512], f32, tag="rtl", name="rtl")
                        # rows 0:32 / 96:128 are dummy-inits (lanes unread)
                        nc.sync.dma_start(out=rtl[0:32], in_=ntmp[0:32])
                        nc.sync.dma_start(out=rtl[32:64], in_=ntmp[0:32])
                        nc.sync.dma_start(out=rtl[64:96], in_=ntmp[96:128])
                        nc.sync.dma_start(out=rtl[96:128], in_=ntmp[96:128])
                        hst = smp.tile([128, 512], bf16, tag="hst",
                                       name="hst")
                        nc.vector.tensor_mul(hst, pv, rtl)
                        ro2 = (g % 2) * 64
                        nc.sync.dma_start(
                            out=hidT[ch][ro2:ro2 + 64,
                                         qh * 512:(qh + 1) * 512],
                            in_=hst[32:96])
                # tail: remaining output projection
                for mq in range(4, 8):
                    out_proj(mq)

    nc.compile()
    return nc


def _make_in_maps(inputs):
    import ml_dtypes

    bf16 = ml_dtypes.bfloat16
    q = np.asarray(inputs["q"], dtype=np.float32)
    k = np.asarray(inputs["k"], dtype=np.float32)
    v = np.asarray(inputs["v"], dtype=np.float32)
    k_b = np.asarray(inputs["k_b"], dtype=np.float32)
    mask = np.asarray(inputs["mask"], dtype=np.int32)
    sw = np.asarray(inputs["scale_w"], dtype=np.float32)
    Wb = np.asarray(inputs["Wb"], dtype=np.float32)
    bb = np.asarray(inputs["bb"], dtype=np.float32)
    Ww = np.asarray(inputs["Ww"], dtype=np.float32)
    bw = np.asarray(inputs["bw"], dtype=np.float32)

    WbT = np.ascontiguousarray(Wb.T).astype(bf16)
    WwT = np.ascontiguousarray(Ww.T).astype(bf16)
    bwb = bw[None, :].astype(bf16)
    ones = np.ones((1, L), dtype=bf16)

    per_batch = {}
    for b in range(B):
        kT = np.ascontiguousarray(k[b].T + bb[:, None]).astype(bf16)
        kbT = np.ascontiguousarray(k_b[b].T).astype(bf16)
        # vmm: [128, kc(16) x h(8) x two(2) x d(32)]
        # h even: [mask | v*mask];  h odd: [v*mask | mask]
        v4 = v[b].reshape(16, 128, NH, DH)
        mk = mask[b].reshape(16, 128).astype(np.float32)
        vm = v4 * mk[:, :, None, None]
        vmm = np.empty((16, 128, NH, 2, DH), dtype=np.float32)
        for h in range(NH):
            vmm[:, :, h, 1 - h % 2, :] = vm[:, :, h, :]
            vmm[:, :, h, h % 2, :] = mk[:, :, None]
        vmm = np.ascontiguousarray(
            vmm.transpose(1, 0, 2, 3, 4).reshape(128, 8192)).astype(bf16)
        per_batch[b] = (kT, kbT, vmm)

    in_maps = []
    for c in range(NCORES):
        b, qs = c // 2, c % 2
        kT, kbT, vmm = per_batch[b]
        qc = q[b, qs * LQ:(qs + 1) * LQ, :]  # [1024, 256]
        swc = sw[:, qs * LQ:(qs + 1) * LQ] * ISQ  # [NH, 1024]
        # scale per (head, query), then transpose to [dims, queries]
        qscaled = qc * np.repeat(swc.T, DH, axis=1)  # [1024, 256]
        qsT = np.ascontiguousarray(qscaled.T).astype(bf16)  # [256, 1024]
        # 64-row-shifted copy within each 128-dim block (kc-parity trick)
        qsT2 = np.ascontiguousarray(
            qsT.reshape(2, 2, 64, LQ)[:, ::-1].reshape(H, LQ))
        in_maps.append({
            "qsT": qsT, "qsT2": qsT2, "kT": kT, "kbT": kbT, "vmm": vmm,
            "WbT": WbT, "WwT": WwT, "bwb": bwb, "ones": ones,
        })
    return in_maps


def run_sharded(inputs, trace=False, tmpdir=None):
    from concourse import bass_utils
    from concourse.bass_utils import run_bass_kernel_spmd

    if trace:
        _install_ntff_hook()
        bass_utils.upload_artifacts = lambda d: d
    nc = _build()
    in_maps = _make_in_maps(inputs)
    res = run_bass_kernel_spmd(nc, in_maps, list(range(NCORES)),
                               trace=trace, tmpdir=tmpdir)
    out = np.empty((B, L, H), dtype=np.float32)
    for c in range(NCORES):
        b, qs = c // 2, c % 2
        out[b, qs * LQ:(qs + 1) * LQ, :] = res.results[c]["out"]
    return out, res


def kernel(**inputs):
    out, _ = run_sharded(inputs, trace=False)
    return out


def _install_ntff_hook():
    """Provide antenv.axon_hooks (absent in this image) so trace=True works."""
    import contextlib
    import ctypes
    import types

    import antenv

    if hasattr(antenv, "axon_hooks"):
        return
    mod = types.ModuleType("antenv.axon_hooks")
    _hook = [None]
    mod.set_axon_ntff_profile_hook = lambda h: _hook.__setitem__(0, h)
    mod.get_axon_ntff_profile_hook = lambda: _hook[0]
    antenv.axon_hooks = mod
    sys.modules["antenv.axon_hooks"] = mod

    lib = ctypes.CDLL("/opt/axon/libaxon_pjrt.so")
    if not hasattr(lib, "axon_start_nrt_profile"):
        return
    lib.axon_start_nrt_profile.argtypes = [ctypes.POINTER(ctypes.c_int64),
                                           ctypes.c_size_t]
    lib.axon_start_nrt_profile.restype = ctypes.c_int64
    lib.axon_stop_nrt_profile.argtypes = [ctypes.c_char_p]
    lib.axon_stop_nrt_profile.restype = ctypes.c_int64

    @contextlib.contextmanager
    def _profile(output_dir, device_ids):
        import jax

        jax.devices()
        if device_ids:
            ids = (ctypes.c_int64 * len(device_ids))(*device_ids)
            rc = lib.axon_start_nrt_profile(ids, len(device_ids))
        else:
            rc = lib.axon_start_nrt_profile(None, 0)
        if rc != 0:
            raise RuntimeError(f"axon_start_nrt_profile rc={rc}")
        try:
            yield
        finally:
            n = lib.axon_stop_nrt_profile(str(output_dir).encode())
            print(f"profile: {n} file(s) written to {output_dir}",
                  file=sys.stderr)

    mod.set_axon_ntff_profile_hook(_profile)


# revision 23
# speedup vs baseline: 1.0834x; 1.0834x over previous
"""Trainium2 Bass kernel for AuxiliaryMultiHeadedAttention.

Reference computation (B=4, L=2048, H=256, NH=8, DH=32):
    kb   = split_heads(k_b @ Wb.T + bb)
    corr = (qh @ kh^T + qh @ kb^T) / sqrt(DH) * scale_w[h, q]
    corr = where(mask==0, -1e9, corr);  prob = softmax(corr)
    out  = merge_heads(prob @ vh) @ Ww.T + bw

Kernel strategy (8 NeuronCores):
    Shard (batch, query-half): core c -> batch c//2, queries (c%2)*1024..+1024.
    Host marshals layouts (pre-transposed bf16 k^T / k_b^T / W^T, the
    mask-interleaved V operand, scale_w chunks) so the device does zero
    layout shuffling; device computes both GEMMs, QK^T, softmax, PV and
    the output projection.
    Each core:
      keffT = (k + k_b @ Wb.T + bb)^T  [dims, keys]  bf16 (dual QK^T folded)
      qsT   = (q * scale_w/sqrt(DH))^T via DMA-xbar  [dims, queries] bf16
      S^T   = keffT_h^T @ qsT_h  (bf16 MMs, 2 heads row-tiled, fp32 psum;
              kc pairs alternate PE row groups via 64-row-shifted copies)
      P^T   = exp(S^T): split between ACT (exact exp, bf16 out) and DVE
              (Schraudolph: int16(A*x+B) bitcast to bf16, one tensor_scalar)
      PV with lhsT [m|v_h] / [v_h|m] (m = mask: masks numerator and
              denominator) -> psum rows [den0|O0|O1|den1]
      hidT  = O * recip(den)  (full-partition recip + mul, DMA realign)
      out   = hidT^T @ WwT + bw (bias via rank-1 matmul), query-half-outer
              loop lets the first half's projection overlap the second half.
    Host concatenates the 8 [1024, 256] slices.
"""

import sys

if "/opt/trn_rl_repo" not in sys.path:
    sys.path.insert(0, "/opt/trn_rl_repo")

import math

import numpy as np

B, L, H, NH, DH = 4, 2048, 256, 8, 32
LQ = 1024  # queries per core
NCORES = 8
ISQ = 1.0 / math.sqrt(DH)

# Schraudolph exp for bf16 target: bf16bits(exp(x)) ~ int16(A16*x + B16)
A16 = 128.0 / math.log(2.0)
C_OFF = 5.5
B16 = 127.0 * 128.0 - C_OFF


def _build():
    import concourse.bass as bass  # noqa: F401
    import concourse.mybir as mybir
    import concourse.tile as tile
    from concourse import bacc

    f32 = mybir.dt.float32
    i16 = mybir.dt.int16
    bf16 = mybir.dt.bfloat16
    Exp = mybir.ActivationFunctionType.Exp

    nc = bacc.Bacc("TRN2", target_bir_lowering=False, debug=False, num_devices=NCORES)

    qsT_d = nc.dram_tensor("qsT", [H, LQ], bf16, kind="ExternalInput")
    qsT2_d = nc.dram_tensor("qsT2", [H, LQ], bf16, kind="ExternalInput")
    kT_d = nc.dram_tensor("kT", [H, L], bf16, kind="ExternalInput")
    kbT_d = nc.dram_tensor("kbT", [H, L], bf16, kind="ExternalInput")
    vmm_d = nc.dram_tensor("vmm", [128, 8192], bf16, kind="ExternalInput")
    WbT_d = nc.dram_tensor("WbT", [H, H], bf16, kind="ExternalInput")
    WwT_d = nc.dram_tensor("WwT", [H, H], bf16, kind="ExternalInput")
    bwb_d = nc.dram_tensor("bwb", [1, H], bf16, kind="ExternalInput")
    ones_d = nc.dram_tensor("ones", [1, L], bf16, kind="ExternalInput")
    out_d = nc.dram_tensor("out", [LQ, H], f32, kind="ExternalOutput")

    copy_flip = [0]

    with tile.TileContext(nc) as tc:
        with (
            tc.tile_pool(name="persist", bufs=1) as pp,
            tc.tile_pool(name="pt", bufs=4) as ptp,
            tc.tile_pool(name="small", bufs=2) as smp,
        ):
            # ---------------- persistent SBUF tensors ----------------
            keffT = [pp.tile([128, L], bf16, tag=f"keffT{g}", name=f"keffT{g}")
                     for g in range(2)]
            keffT2 = [pp.tile([128, L], bf16, tag=f"keffT2_{g}",
                              name=f"keffT2_{g}") for g in range(2)]
            qsT = [pp.tile([128, LQ], bf16, tag=f"qsT{g}", name=f"qsT{g}")
                   for g in range(2)]
            qsT2 = [pp.tile([128, LQ], bf16, tag=f"qsT2_{g}", name=f"qsT2_{g}")
                    for g in range(2)]
            # per (key-chunk, head): [m|v_h] (h even) / [v_h|m] (h odd);
            # m = mask column (masks numerator and denominator)
            vmm = pp.tile([128, 16 * NH * 64], bf16, tag="vmm")
            hidT = [pp.tile([128, LQ], bf16, tag=f"hidT{g}", name=f"hidT{g}")
                    for g in range(2)]
            outsb = pp.tile([128, 8 * H], f32, tag="outsb")

            kbTt = [pp.tile([128, L], bf16, tag=f"kbTt{e}", name=f"kbTt{e}")
                    for e in range(2)]
            kTt = [pp.tile([128, L], bf16, tag=f"kTt{e}", name=f"kTt{e}")
                   for e in range(2)]
            WbTt = [pp.tile([128, H], bf16, tag=f"WbTt{e}", name=f"WbTt{e}")
                    for e in range(2)]
            WwTt = [pp.tile([128, H], bf16, tag=f"WwTt{g}", name=f"WwTt{g}")
                    for g in range(2)]
            bwb = pp.tile([1, H], bf16, tag="bwb")
            ones = pp.tile([1, L], bf16, tag="ones")
            wsc = pp.tile([128, 256], bf16, tag="wsc")
            zf = pp.tile([128, 64], bf16, tag="zf")

            def pcopy(dst, src):
                # alternate psum->sbuf evacuation between DVE and ACT
                if copy_flip[0] % 2 == 0:
                    nc.vector.tensor_copy(dst, src)
                else:
                    nc.scalar.copy(dst, src)
                copy_flip[0] += 1

            # ---------------- staging loads ----------------
            # 3 DMA queues (sync/scalar HWDGE + gpsimd SWDGE), ordered by
            # criticality; vmm (largest) is chunked so it trails into the
            # main loop (PV consumes it in kc order).
            vmv = vmm.rearrange("p (c f) -> p c f", c=4)
            # warm-up operand needs no load
            nc.vector.memset(wsc, 0.5)
            nc.vector.memset(zf, 0.0)
            # gpsimd: kbT ec0 (keff critical path); keffT2 shifts + vmm
            # kc 8-15 queued after the keff GEMM below
            nc.gpsimd.dma_start(out=kbTt[0], in_=kbT_d[0:128, :])
            # scalar: kbT ec1, q tiles, vmm kc 0-7 trailing
            nc.scalar.dma_start(out=kbTt[1], in_=kbT_d[128:256, :])
            for dc in range(2):
                nc.scalar.dma_start(out=qsT[dc],
                                    in_=qsT_d[dc * 128:(dc + 1) * 128, :])
                nc.scalar.dma_start(out=qsT2[dc],
                                    in_=qsT2_d[dc * 128:(dc + 1) * 128, :])
            nc.scalar.dma_start(out=vmv[:, 0, :], in_=vmm_d[:, 0:2048])
            nc.scalar.dma_start(out=vmv[:, 1, :], in_=vmm_d[:, 2048:4096])
            # sync: keff deps first, tail-only tensors last
            nc.sync.dma_start(out=WbTt[0], in_=WbT_d[0:128, :])
            nc.sync.dma_start(out=WbTt[1], in_=WbT_d[128:256, :])
            for ec in range(2):
                nc.sync.dma_start(out=kTt[ec],
                                  in_=kT_d[ec * 128:(ec + 1) * 128, :])
            nc.sync.dma_start(out=WwTt[0], in_=WwT_d[0:128, :])
            nc.sync.dma_start(out=WwTt[1], in_=WwT_d[128:256, :])
            nc.sync.dma_start(out=bwb, in_=bwb_d[:, :])
            nc.sync.dma_start(out=ones, in_=ones_d[:, :])

            # ---------------- keff GEMM ----------------
            # bb is pre-added into kT on the host, so no bias matmuls:
            # keffT = WbT.T @ kbT (psum) + (k.T + bb) via fused evac add.
            with tc.tile_pool(name="pkeff", bufs=4, space="PSUM") as pkf:
                # PE warm-up burst: dense dummy matmuls on a memset scratch
                # release the HAM clock gate (~3.4us of sustained PE
                # activity) before keff/QK need full rate.
                for w in range(32):
                    pwt = (w % 8) // 2
                    pw = pkf.tile([128, 512], f32, tag="pk", name="pkw")
                    nc.tensor.matmul(pw[:, 0:256],
                                     lhsT=wsc[:, 0:128],
                                     rhs=wsc[:, 0:256],
                                     start=True, stop=True,
                                     skip_group_check=True)
                # keff GEMM, one psum tile per 512-key chunk so the DVE
                # evacuation never serializes the PE stream (tile-granular
                # dependency tracking)
                for dc in range(2):
                    for ns in range(4):
                        co = slice(ns * 512, (ns + 1) * 512)
                        pk = pkf.tile([128, 512], f32, tag="pk",
                                      name=f"pk{dc}_{ns}")
                        for ec in range(2):
                            nc.tensor.matmul(
                                pk,
                                lhsT=WbTt[ec][:, dc * 128:(dc + 1) * 128],
                                rhs=kbTt[ec][:, co],
                                start=(ec == 0), stop=(ec == 1))
                        # evac with fused +(k+bb) add; 64-row shift per ns
                        # so QK for kc 4ns.. can start immediately
                        nc.vector.tensor_add(keffT[dc][:, co], pk,
                                             kTt[dc][:, co])
                        nc.gpsimd.dma_start(out=keffT2[dc][0:64, co],
                                            in_=keffT[dc][64:128, co])
                        nc.gpsimd.dma_start(out=keffT2[dc][64:128, co],
                                            in_=keffT[dc][0:64, co])
                # vmm kc 8-15 trail on the gpsimd queue
                nc.gpsimd.dma_start(out=vmv[:, 2, :], in_=vmm_d[:, 4096:6144])
                nc.gpsimd.dma_start(out=vmv[:, 3, :], in_=vmm_d[:, 6144:8192])

            # ---------------- main attention loop ----------------
            # query-half outer so the first half's output projection can
            # overlap the second half's attention.
            # group g: heads (2g, 2g+1); chunk ch = g//2.
            # kc processed in pairs with alternating PE row groups (via the
            # 64-row-shifted tile copies): the pair's 4 QK matmuls occupy 4
            # distinct 32-row groups and stream concurrently.
            with (
                tc.tile_pool(name="pst", bufs=3, space="PSUM") as pst,
                tc.tile_pool(name="ppv", bufs=2, space="PSUM") as ppv,
            ):
                def out_proj(mq):
                    # out[mq] = hidT^T @ WwT + bw (bias via rank-1 matmul)
                    po = pst.tile([128, 1024], f32, tag="st", name="po")
                    nc.tensor.matmul(po[:, 0:256],
                                     lhsT=ones[0:1, mq * 128:(mq + 1) * 128],
                                     rhs=bwb[0:1, :],
                                     start=True, stop=False)
                    for gg in range(2):
                        nc.tensor.matmul(
                            po[:, 0:256],
                            lhsT=hidT[gg][:, mq * 128:(mq + 1) * 128],
                            rhs=WwTt[gg],
                            start=False, stop=(gg == 1))
                    pcopy(outsb[:, mq * H:(mq + 1) * H], po[:, 0:256])
                    if mq % 2 == 1:
                        cs2 = slice(mq - 1, mq + 1)
                        nc.sync.dma_start(
                            out=out_d.rearrange("(c p) e -> p c e",
                                                p=128)[:, cs2, :],
                            in_=outsb.rearrange("p (c e) -> p c e",
                                                c=8)[:, cs2, :])

                def pv_mms(pv, g, kc2, pt, last):
                    # PV: h even lhsT=[m|v] -> rows [den|O];
                    #     h odd  lhsT=[v|m] -> rows [O|den]
                    for t in range(2):
                        h = 2 * g + t
                        nc.tensor.matmul(
                            pv[64 * t:64 * t + 64, :],
                            lhsT=vmm[:, (kc2 * NH + h) * 64:
                                     (kc2 * NH + h) * 64 + 64],
                            rhs=pt[:, t * 512:(t + 1) * 512],
                            tile_position=(0, 64 * t),
                            start=(kc2 == 0), stop=last,
                            skip_group_check=True)

                for qh in range(2):
                    for g in range(4):
                        ch = g // 2
                        pv = ppv.tile([128, 512], f32, tag="pv",
                                      name=f"pv{qh}_{g}")
                        # software-pipelined by two kc: PV(k) is issued
                        # after QK(k+2)+exp(k+2) so the (dependency-stalled)
                        # PV matmul never head-of-line-blocks later QKs
                        # in the PE's in-order queue.
                        pend = []
                        for kc2 in range(16):
                            idx = ((qh * 4 + g) * 16 + kc2) // 2
                            par = kc2 % 2
                            kket = keffT[ch] if par == 0 else keffT2[ch]
                            qqt = qsT[ch] if par == 0 else qsT2[ch]
                            rbase = ((g % 2) * 64 if par == 0
                                     else (1 - g % 2) * 64)
                            st = pst.tile([128, 1024], f32, tag="st",
                                          name=f"st{par}")
                            for t in range(2):
                                ro = rbase + t * 32
                                nc.tensor.matmul(
                                    st[:, t * 512:(t + 1) * 512],
                                    lhsT=kket[ro:ro + 32,
                                              kc2 * 128:(kc2 + 1) * 128],
                                    rhs=qqt[ro:ro + 32,
                                            qh * 512:(qh + 1) * 512],
                                    tile_position=(ro, 0),
                                    start=True, stop=True)
                            # exp: odd kc -> ACT; even kc -> DVE
                            # (Schraudolph), except every 16th even also
                            # ACT (balance: 72 ACT / 56 DVE)
                            if par == 1 or idx % 16 == 5:
                                pt = ptp.tile([128, 1024], bf16,
                                              tag="ptA", name="ptA")
                                nc.scalar.activation(pt, st, Exp)
                            else:
                                pti = ptp.tile([128, 1024], i16,
                                               tag="ptD", name="ptD")
                                nc.vector.tensor_scalar(
                                    out=pti, in0=st, scalar1=A16,
                                    scalar2=B16,
                                    op0=mybir.AluOpType.mult,
                                    op1=mybir.AluOpType.add)
                                pt = pti.bitcast(bf16)
                            pend.append((kc2, pt))
                            if len(pend) > 2:
                                k0, p0 = pend.pop(0)
                                pv_mms(pv, g, k0, p0, False)
                            # interleave qh=0's output projection into
                            # qh=1/g=0 so only mq 4-7 remain in the tail
                            if qh == 1 and g == 0 and kc2 in (4, 6, 8, 10):
                                out_proj((kc2 - 4) // 2)
                        for k0, p0 in pend:
                            pv_mms(pv, g, k0, p0, k0 == 15)
                        # normalize: pv rows = [den0 | O0 | O1 | den1].
                        # Full 128-partition ops; unused lanes compute
                        # garbage, unread.
                        ntmp = smp.tile([128, 512], f32, tag="ntmp",
                                        name="ntmp")
                        nc.vector.reciprocal_approx_fast(ntmp, pv)
                        rtl = smp.tile([128, 512], f32, tag="rtl", name="rtl")
                        # rows 0:32 / 96:128 are dummy-inits (lanes unread)
                        nc.sync.dma_start(out=rtl[0:32], in_=ntmp[0:32])
                        nc.sync.dma_start(out=rtl[32:64], in_=ntmp[0:32])
                        nc.sync.dma_start(out=rtl[64:96], in_=ntmp[96:128])
                        nc.sync.dma_start(out=rtl[96:128], in_=ntmp[96:128])
                        hst = smp.tile([128, 512], bf16, tag="hst",
                                       name="hst")
                        nc.vector.tensor_mul(hst, pv, rtl)
                        ro2 = (g % 2) * 64
                        nc.sync.dma_start(
                            out=hidT[ch][ro2:ro2 + 64,
                                         qh * 512:(qh + 1) * 512],
                            in_=hst[32:96])
                # tail: remaining output projection
                for mq in range(4, 8):
                    out_proj(mq)

    nc.compile()
    return nc


def _make_in_maps(inputs):
    import ml_dtypes

    bf16 = ml_dtypes.bfloat16
    q = np.asarray(inputs["q"], dtype=np.float32)
    k = np.asarray(inputs["k"], dtype=np.float32)
    v = np.asarray(inputs["v"], dtype=np.float32)
    k_b = np.asarray(inputs["k_b"], dtype=np.float32)
    mask = np.asarray(inputs["mask"], dtype=np.int32)
    sw = np.asarray(inputs["scale_w"], dtype=np.float32)
    Wb = np.asarray(inputs["Wb"], dtype=np.float32)
    bb = np.asarray(inputs["bb"], dtype=np.float32)
    Ww = np.asarray(inputs["Ww"], dtype=np.float32)
    bw = np.asarray(inputs["bw"], dtype=np.float32)

    WbT = np.ascontiguousarray(Wb.T).astype(bf16)
    WwT = np.ascontiguousarray(Ww.T).astype(bf16)
    bwb = bw[None, :].astype(bf16)
    ones = np.ones((1, L), dtype=bf16)

    per_batch = {}
    for b in range(B):
        kT = np.ascontiguousarray(k[b].T + bb[:, None]).astype(bf16)
        kbT = np.ascontiguousarray(k_b[b].T).astype(bf16)
        # vmm: [128, kc(16) x h(8) x two(2) x d(32)]
        # h even: [mask | v*mask];  h odd: [v*mask | mask]
        v4 = v[b].reshape(16, 128, NH, DH)
        mk = mask[b].reshape(16, 128).astype(np.float32)
        vm = v4 * mk[:, :, None, None]
        vmm = np.empty((16, 128, NH, 2, DH), dtype=np.float32)
        for h in range(NH):
            vmm[:, :, h, 1 - h % 2, :] = vm[:, :, h, :]
            vmm[:, :, h, h % 2, :] = mk[:, :, None]
        vmm = np.ascontiguousarray(
            vmm.transpose(1, 0, 2, 3, 4).reshape(128, 8192)).astype(bf16)
        per_batch[b] = (kT, kbT, vmm)

    in_maps = []
    for c in range(NCORES):
        b, qs = c // 2, c % 2
        kT, kbT, vmm = per_batch[b]
        qc = q[b, qs * LQ:(qs + 1) * LQ, :]  # [1024, 256]
        swc = sw[:, qs * LQ:(qs + 1) * LQ] * ISQ  # [NH, 1024]
        # scale per (head, query), then transpose to [dims, queries]
        qscaled = qc * np.repeat(swc.T, DH, axis=1)  # [1024, 256]
        qsT = np.ascontiguousarray(qscaled.T).astype(bf16)  # [256, 1024]
        # 64-row-shifted copy within each 128-dim block (kc-parity trick)
        qsT2 = np.ascontiguousarray(
            qsT.reshape(2, 2, 64, LQ)[:, ::-1].reshape(H, LQ))
        in_maps.append({
            "qsT": qsT, "qsT2": qsT2, "kT": kT, "kbT": kbT, "vmm": vmm,
            "WbT": WbT, "WwT": WwT, "bwb": bwb, "ones": ones,
        })
    return in_maps


def run_sharded(inputs, trace=False, tmpdir=None):
    from concourse import bass_utils
    from concourse.bass_utils import run_bass_kernel_spmd

    if trace:
        _install_ntff_hook()
        bass_utils.upload_artifacts = lambda d: d
    nc = _build()
    in_maps = _make_in_maps(inputs)
    res = run_bass_kernel_spmd(nc, in_maps, list(range(NCORES)),
                               trace=trace, tmpdir=tmpdir)
    out = np.empty((B, L, H), dtype=np.float32)
    for c in range(NCORES):
        b, qs = c // 2, c % 2
        out[b, qs * LQ:(qs + 1) * LQ, :] = res.results[c]["out"]
    return out, res


def kernel(**inputs):
    out, _ = run_sharded(inputs, trace=False)
    return out


def _install_ntff_hook():
    """Provide antenv.axon_hooks (absent in this image) so trace=True works."""
    import contextlib
    import ctypes
    import types

    import antenv

    if hasattr(antenv, "axon_hooks"):
        return
    mod = types.ModuleType("antenv.axon_hooks")
    _hook = [None]
    mod.set_axon_ntff_profile_hook = lambda h: _hook.__setitem__(0, h)
    mod.get_axon_ntff_profile_hook = lambda: _hook[0]
    antenv.axon_hooks = mod
    sys.modules["antenv.axon_hooks"] = mod

    lib = ctypes.CDLL("/opt/axon/libaxon_pjrt.so")
    if not hasattr(lib, "axon_start_nrt_profile"):
        return
    lib.axon_start_nrt_profile.argtypes = [ctypes.POINTER(ctypes.c_int64),
                                           ctypes.c_size_t]
    lib.axon_start_nrt_profile.restype = ctypes.c_int64
    lib.axon_stop_nrt_profile.argtypes = [ctypes.c_char_p]
    lib.axon_stop_nrt_profile.restype = ctypes.c_int64

    @contextlib.contextmanager
    def _profile(output_dir, device_ids):
        import jax

        jax.devices()
        if device_ids:
            ids = (ctypes.c_int64 * len(device_ids))(*device_ids)
            rc = lib.axon_start_nrt_profile(ids, len(device_ids))
        else:
            rc = lib.axon_start_nrt_profile(None, 0)
        if rc != 0:
            raise RuntimeError(f"axon_start_nrt_profile rc={rc}")
        try:
            yield
        finally:
            n = lib.axon_stop_nrt_profile(str(output_dir).encode())
            print(f"profile: {n} file(s) written to {output_dir}",
                  file=sys.stderr)

    mod.set_axon_ntff_profile_hook(_profile)
